# revision 14
# baseline (speedup 1.0000x reference)
"""Trainium2 Bass kernel for the DifferentiableLayer (moe_routing) problem.

Computes out[b, o] = sum_{i,k} onehot(argmax_k(weights+gumbel))[o,i,k] * ops(x)[b,i,k]
where ops(x) = [x, sin x, cos x, tanh x, x^2, relu x] along k.

Forward value of the straight-through hard gumbel-softmax is exactly the
one-hot of argmax_k(weights + gumbel) (softmax is monotonic).

Structure: the host ships every tensor as the exact fp16 SBUF image the
kernel wants (partition-major, fully contiguous DMA), with the
contraction index i on partitions and the w/g layout [i%128, (i//128, k, o)]
so each per-chunk DMA-accumulate is a single contiguous run per
partition (cheap SWDGE descriptor generation) and every VectorE op runs
on contiguous 16-bit slabs:
  - s = w + g via SWDGE DMA accumulate, one i-chunk at a time
  - max_k via a 5-op tensor_tensor max tree over the six [128, o] slabs
  - P^T[it, k, o] = (s == m) in one broadcast compare per i-chunk
    (m broadcast over the middle k axis, innermost o stays contiguous)
  - sin/cos: xs = x/(2pi) shared prescale, then one scalar_tensor_tensor
    fold each (v = [x>=t] - xs), then ACT Sin(2pi*v + bias):
      sin(x) = Sin(2pi*([x>=0]     - x/2pi) - pi)
      cos(x) = Sin(2pi*([x>=-pi/2] - x/2pi) - 3pi/2)
    (the handful of |x| past the Sin table edge contribute O(1e-4) rel)
  - out^T[o, b] += P^T_k . ops_k^T: 96 accumulating N=512 fp16 matmuls
    at the 1 col/cycle PE streaming roofline (~216ns each warm)
fp16 for w+g keeps the argmax flip rate ~3e-4 (~3e-3 rel err measured
vs the fp32 reference; tolerance 2e-2).

A burst of N=128 scratch matmuls at t=0 warms the PE HAM clock gate
(4/8 -> 8/8) before the first real matmul issues.

Sharding: 4 batch shards x 2 out-feature shards over 8 cores.

The 64-byte engine instruction structs have a single sync-wait slot, so
cross-engine waits that would stack on one instruction are absorbed by
dependency-carrying nops, and a post-pass strips waits that are provably
dominated by an earlier wait on the same in-order queue.
"""

import numpy as np

from concourse import bass, mybir, tile
from concourse.bass import _add_dep_helper
from concourse.bass_utils import run_bass_kernel_spmd

F16 = mybir.dt.float16
F32 = mybir.dt.float32
AF = mybir.ActivationFunctionType
ALU = mybir.AluOpType

B, I, O, K = 4096, 512, 512, 6
NB, NO = 4, 2                # batch shards x out-feature shards
BL, OL = B // NB, O // NO    # 1024, 256 per core
NCORES = NB * NO

NIT = I // 128               # 4 i-chunks (contraction tiles)
NOT = OL // 128              # 2 o-tiles (psum partition tiles)
NBC = BL // 512              # 2 b-chunks (psum free tiles)
NDUMMY = 32                  # PE warm-up matmuls (N=128, ~107ns each cold)

_PI = float(np.pi)

_ENGINE_SEM = {
    "EngineType.PE": "PE",
    "EngineType.Activation": "Activation",
    "EngineType.DVE": "DVE",
}


def _strip_redundant_waits(nc: bass.Bass) -> None:
    """Drop sync waits that are dominated by an earlier wait on the same
    in-order engine queue, or (for PE/ACT/DVE) implied by the engine's own
    completion-semaphore order.  Needed because the HW instruction structs
    hold a single wait command."""
    import re

    seen = {}      # sem name -> cumulative update count
    observed = {}  # (engine, sem name) -> max wait value already waited for
    for bb in nc.main_func.blocks:
        for ins in bb.instructions:
            si = ins.sync_info
            if si is None:
                continue
            eng = str(ins.engine)
            if len(si.on_wait) >= 2:
                own = _ENGINE_SEM.get(eng)
                keep = []
                for w in si.on_wait:
                    if observed.get((eng, w.ant_name), -1) >= w.wait_value:
                        continue
                    if (
                        own is not None
                        and re.fullmatch(rf"{own}_\d+", w.ant_name)
                        and seen.get(w.ant_name, 0) >= w.wait_value
                    ):
                        continue
                    keep.append(w)
                if len(keep) != len(si.on_wait):
                    si.on_wait = keep
            for w in si.on_wait:
                key = (eng, w.ant_name)
                if observed.get(key, -1) < w.wait_value:
                    observed[key] = w.wait_value
            for u in si.on_update:
                if u.update_value is not None:
                    seen[u.ant_name] = seen.get(u.ant_name, 0) + u.update_value
    return


def _build_program() -> bass.Bass:
    nc = bass.Bass()

    # All inputs are pre-swizzled SBUF images: [128 partitions, free elems].
    xt_in = nc.declare_dram_parameter("xt", [128, NIT * BL], F16, isOutput=False)
    w_in = nc.declare_dram_parameter("w", [128, NIT * K * OL], F16, isOutput=False)
    g_in = nc.declare_dram_parameter("g", [128, NIT * K * OL], F16, isOutput=False)
    out_ext = nc.declare_dram_parameter("out", [128, NOT * BL], F16, isOutput=True)

    def dep(a, b, why):
        _add_dep_helper(a.ins, b.ins, sync=True, reason=why)

    with tile.TileContext(nc) as tc:
        with (
            tc.tile_pool(name="big", bufs=1) as big,
            tc.tile_pool(name="psum_out", bufs=1, space="PSUM") as pout,
        ):
            # ---- SBUF tiles ----
            xt_sb = big.tile([128, NIT * BL], F16)          # [p, (it, b)]
            xs_sb = big.tile([128, NIT * BL], F16)          # x/(2pi)
            s_sb = big.tile([128, NIT * K * OL], F16)       # [p, (it, k, o)] = w+g
            g_sb = big.tile([128, NIT * K * OL], F16)       # g landing buffer
            m_sb = big.tile([128, NIT * OL], F16)           # [p, (it, o)]
            pT_sb = big.tile([128, NIT * K * OL], F16)      # [p, (it, k, o)] one-hot
            tre_sb = big.tile([128, NIT * 4 * OL], F16)     # max-tree temps
            wrap_sb = big.tile([128, 2 * NIT * BL], F16)    # [p, (f, it, b)]
            ops_sb = big.tile([128, 5 * NIT * BL], F16)     # [p, (q, it, b)]
            out_sb = big.tile([128, NOT * BL], F16)         # [p, (ot, b)]
            scr_sb = big.tile([128, 128], F16)              # PE warm-up scratch
            b_sin = big.tile([128, 1], F32)                 # -pi
            b_cos = big.tile([128, 1], F32)                 # -3pi/2
            b_scl = big.tile([128, 1], F32)                 # 2pi

            xt_f = xt_sb[:]                                  # [128, 4096]
            xt_v = xt_f.rearrange("p (it b) -> p it b", it=NIT)
            xs_f = xs_sb[:]
            s_v = s_sb[:].rearrange("p (it k o) -> p it k o", k=K, it=NIT)
            s_c = s_sb[:].rearrange("p (it ko) -> p it ko", it=NIT)
            g_c = g_sb[:].rearrange("p (it ko) -> p it ko", it=NIT)
            m_v = m_sb[:].rearrange("p (it o) -> p it o", it=NIT)
            pT_v = pT_sb[:].rearrange("p (it k o) -> p it k o", k=K, it=NIT)
            tre_v = tre_sb[:].rearrange("p (it t o) -> p it t o", it=NIT, t=4)
            wrap_f = wrap_sb[:]                              # [128, 2*4096]
            ops_v = ops_sb[:].rearrange("p (q it b) -> p q it b", q=5, it=NIT)
            out_v = out_sb[:].rearrange("p (ot b) -> p ot b", ot=NOT)

            # ---- PSUM tiles ----
            po = []
            for i in range(NOT * NBC):
                po_tile = pout.tile([128, 512], F32, tag=f"po{i}")
                po.append(po_tile)
            pscr = pout.tile([128, 512], F32, tag="pscr")

            # ---- constants / warm-up ----
            scr_ms = nc.gpsimd.memset(scr_sb[:], 0.0)
            ms_sin = nc.gpsimd.memset(b_sin[:], -_PI)
            ms_cos = nc.gpsimd.memset(b_cos[:], -1.5 * _PI)
            ms_scl = nc.gpsimd.memset(b_scl[:], 2.0 * _PI)
            npe = nc.tensor.nop()
            dep(npe, scr_ms, "absorb scratch memset wait on PE")
            for d in range(NDUMMY):
                sl = (d % 4) * 128
                nc.tensor.matmul(
                    pscr[:, sl : sl + 128], scr_sb[:], scr_sb[:],
                    start=True, stop=True,
                )

            # ---- DMA loads, all on the SP HWDGE queue; g accumulated
            #      onto w via SWDGE CCE add (1 contiguous run/partition) ----
            xt_dram = xt_in[:].rearrange("p (h b) -> p h b", h=2)
            xt_hv = xt_f.rearrange("p (h b) -> p h b", h=2)
            w_dram = w_in[:].rearrange("p (it ko) -> p it ko", it=NIT)
            g_dram = g_in[:].rearrange("p (it ko) -> p it ko", it=NIT)

            wd, gd = [], []
            tail_deps = [scr_ms, ms_sin, ms_cos, ms_scl]
            # w/g interleaved on the SP HWDGE ring (FIFO completion order)
            for it in range(NIT):
                w_i = nc.sync.dma_start(out=s_c[:, it], in_=w_dram[:, it])
                g_i = nc.sync.dma_start(out=g_c[:, it], in_=g_dram[:, it])
                wd.append(w_i)
                gd.append(g_i)
                tail_deps.extend([w_i, g_i])
            # x halves on the ACT HWDGE ring (parallel to w/g)
            x_h0 = nc.scalar.dma_start(out=xt_hv[:, 0], in_=xt_dram[:, 0])
            x_h1 = nc.scalar.dma_start(out=xt_hv[:, 1], in_=xt_dram[:, 1])
            xd = [x_h0, x_h0, x_h1, x_h1]   # per-chunk alias (halves)
            tail_deps.extend([x_h0, x_h1])

            # ---- VectorE ----
            half = 2 * BL  # 2048 columns per half

            def hs(base, q, h):
                lo = q * NIT * BL + h * half
                return base[:, lo : lo + half]

            wrapS, wrapC, relu_i, sq_i, eq = {}, {}, {}, {}, {}

            def emit_wrap(h, f, thresh):
                nv = nc.vector.nop()
                dep(nv, xd[2 * h], "absorb x dma wait on DVE")
                tail_deps.append(nv)
                xsl = xt_f[:, h * half : (h + 1) * half]
                t = hs(wrap_f, f, h)
                nc.vector.tensor_scalar(
                    t, xsl, thresh, 2.0 * _PI, op0=ALU.is_ge, op1=ALU.mult
                )
                return nc.vector.tensor_sub(t, t, xsl)

            def emit_relu_sq(h):
                ngp = nc.gpsimd.nop()
                dep(ngp, xd[2 * h], "absorb x dma wait on GpSimd")
                tail_deps.append(ngp)
                xsl = xt_f[:, h * half : (h + 1) * half]
                relu_i[h] = nc.gpsimd.tensor_scalar_max(
                    hs(ops_sb[:], 4, h), xsl, 0.0
                )
                sq_i[h] = nc.gpsimd.tensor_mul(hs(ops_sb[:], 3, h), xsl, xsl)

            def emit_mask(it):
                nv = nc.vector.nop()
                dep(nv, gd[it], "absorb g dma wait on DVE")
                tail_deps.append(nv)
                nc.vector.tensor_add(s_c[:, it], s_c[:, it], g_c[:, it])
                t = tre_v
                nc.vector.tensor_tensor(t[:, it, 0], s_v[:, it, 0], s_v[:, it, 1], op=ALU.max)
                nc.vector.tensor_tensor(t[:, it, 1], s_v[:, it, 2], s_v[:, it, 3], op=ALU.max)
                nc.vector.tensor_tensor(t[:, it, 2], s_v[:, it, 4], s_v[:, it, 5], op=ALU.max)
                nc.vector.tensor_tensor(t[:, it, 3], t[:, it, 0], t[:, it, 1], op=ALU.max)
                nc.vector.tensor_tensor(m_v[:, it], t[:, it, 2], t[:, it, 3], op=ALU.max)
                mb = m_v[:, it].unsqueeze(1).to_broadcast((128, K, OL))
                eq[it] = nc.vector.tensor_tensor(
                    pT_v[:, it], s_v[:, it], mb, op=ALU.is_equal
                )

            emit_relu_sq(0)      # gpsimd, needs x_h0
            emit_relu_sq(1)      # gpsimd, needs x_h1
            wrapS[0] = emit_wrap(0, 0, 0.0)
            wrapC[0] = emit_wrap(0, 1, -0.5 * _PI)
            emit_mask(0)         # needs g0
            emit_mask(1)
            wrapS[1] = emit_wrap(1, 0, 0.0)
            wrapC[1] = emit_wrap(1, 1, -0.5 * _PI)
            emit_mask(2)
            emit_mask(3)

            # ---- ScalarE: transcendentals per half ----
            for b in (ms_sin, ms_cos, ms_scl):
                nsc = nc.scalar.nop()
                dep(nsc, b, "absorb bias memset wait on ACT")
                tail_deps.append(nsc)
            for h in range(2):
                na = nc.scalar.nop()
                dep(na, xd[2 * h], "absorb x dma wait on ACT")
                tail_deps.append(na)
            act = {}
            act[("tanh", 0)] = nc.scalar.activation(
                hs(ops_sb[:], 2, 0), xt_f[:, 0:half], AF.Tanh
            )
            act[("sin", 0)] = nc.scalar.activation(
                hs(ops_sb[:], 0, 0), hs(wrap_f, 0, 0), AF.Sin,
                bias=b_sin[:],
            )
            act[("tanh", 1)] = nc.scalar.activation(
                hs(ops_sb[:], 2, 1), xt_f[:, half : 2 * half], AF.Tanh
            )
            act[("cos", 0)] = nc.scalar.activation(
                hs(ops_sb[:], 1, 0), hs(wrap_f, 1, 0), AF.Sin,
                bias=b_cos[:],
            )
            act[("sin", 1)] = nc.scalar.activation(
                hs(ops_sb[:], 0, 1), hs(wrap_f, 0, 1), AF.Sin,
                bias=b_sin[:],
            )
            act[("cos", 1)] = nc.scalar.activation(
                hs(ops_sb[:], 1, 1), hs(wrap_f, 1, 1), AF.Sin,
                bias=b_cos[:],
            )

            # ---- main matmuls ----
            # mask slot k (reference op order): 0=x 1=sin 2=cos 3=tanh
            # 4=sq 5=relu ; ops_v q: 0=sin 1=cos 2=tanh 3=sq 4=relu
            def rhs_src(k, it, bc):
                if k == 0:
                    return xt_v[:, it, bc * 512 : (bc + 1) * 512]
                return ops_v[:, k - 1, it, bc * 512 : (bc + 1) * 512]

            order = [
                (0, 0), (0, 3),
                (1, 0), (1, 3),
                (0, 1), (1, 1),
                (2, 0), (2, 3),
                (0, 5), (0, 4), (1, 5), (1, 4),
                (0, 2), (1, 2),
                (2, 1),
                (3, 0), (3, 3), (3, 1),
                (2, 5), (2, 4), (3, 5), (3, 4),
                (2, 2), (3, 2),
            ]
            assert len(order) == 6 * NIT
            counts = {}
            xd_absorbed = set()
            last_mm = None
            for it, k in order:
                if k == 0 and it not in xd_absorbed:
                    nx = nc.tensor.nop()
                    dep(nx, xd[it], "absorb x dma wait on PE")
                    xd_absorbed.add(it)
                for ot in range(NOT):
                    for bc in range(NBC):
                        pid = ot * NBC + bc
                        n = counts[pid] = counts.get(pid, 0) + 1
                        lhsT = pT_v[:, it, k, ot * 128 : (ot + 1) * 128]
                        last_mm = nc.tensor.matmul(
                            po[pid][:],
                            lhsT,
                            rhs_src(k, it, bc),
                            start=(n == 1),
                            stop=(n == len(order)),
                        )

            # ---- drain psums (2 on DVE, 2 on ACT — both idle by now) ----
            drains = []
            for ot in range(NOT):
                for bc in range(NBC):
                    pid = ot * NBC + bc
                    dst = out_v[:, ot, bc * 512 : (bc + 1) * 512]
                    if bc == 0:
                        d = nc.vector.tensor_copy(dst, po[pid][:])
                    else:
                        d = nc.scalar.copy(dst, po[pid][:])
                    drains.append(d)
            out_dram = out_ext[:].rearrange("p (ot b) -> p ot b", ot=NOT)
            for ot in range(NOT):
                for d in (drains[ot * NBC], drains[ot * NBC + 1]):
                    ns = nc.sync.nop()
                    dep(ns, d, "absorb drain wait before out dma")
                    tail_deps.append(ns)
                od = nc.sync.dma_start(out=out_dram[:, ot], in_=out_v[:, ot])
                tail_deps.append(od)

            # absorb outstanding completions on the SP queue so the
            # framework's tail drain ends up with only dominated waits
            tail_deps.extend(drains)
            tail_deps.append(last_mm)
            for v in (
                list(wrapS.values()) + list(wrapC.values())
                + list(relu_i.values()) + list(sq_i.values())
                + list(eq.values()) + list(act.values())
            ):
                tail_deps.append(v)
            for d in tail_deps:
                n = nc.sync.nop()
                dep(n, d, "tail wait absorb")

    _strip_redundant_waits(nc)
    return nc


_NC_CACHE = None


def _get_program():
    global _NC_CACHE
    if _NC_CACHE is None:
        _NC_CACHE = _build_program()
    return _NC_CACHE


def _shard_inputs(x, weights, gumbel):
    # x image: ximg[p, it*BL + b] = x[bs*BL + b, it*128 + p]
    xT = np.asarray(x, dtype=np.float32).T.astype(np.float16)   # [I, B]
    # w image: wimg[p, (it, k, o)] = w[o0 + o, it*128 + p, k]
    wT = np.asarray(weights, dtype=np.float32).transpose(2, 1, 0).astype(np.float16)  # [K, I, O]
    gT = np.asarray(gumbel, dtype=np.float32).transpose(2, 1, 0).astype(np.float16)

    def wimg(a, t):
        blk = a[:, :, t * OL : (t + 1) * OL]              # [K, I, OL]
        blk = blk.reshape(K, NIT, 128, OL)                # [K, it, p, o]
        return np.ascontiguousarray(
            blk.transpose(2, 1, 0, 3).reshape(128, NIT * K * OL)
        )

    def ximg(bs):
        blk = xT[:, bs * BL : (bs + 1) * BL]              # [I, BL]
        blk = blk.reshape(NIT, 128, BL)                   # [it, p, b]
        return np.ascontiguousarray(
            blk.transpose(1, 0, 2).reshape(128, NIT * BL)
        )

    wi = [wimg(wT, t) for t in range(NO)]
    gi = [wimg(gT, t) for t in range(NO)]
    xi = [ximg(bs) for bs in range(NB)]
    in_maps = []
    for c in range(NCORES):
        t, bs = divmod(c, NB)
        in_maps.append({"xt": xi[bs], "w": wi[t], "g": gi[t]})
    return in_maps


def _unshard(results):
    out = np.empty((B, O), dtype=np.float32)
    for c in range(NCORES):
        t, bs = divmod(c, NB)
        img = np.asarray(results[c]["out"])               # [128, ot*BL]
        blk = img.reshape(128, NOT, BL).transpose(1, 0, 2).reshape(OL, BL)
        out[bs * BL : (bs + 1) * BL, t * OL : (t + 1) * OL] = (
            blk.T.astype(np.float32)
        )
    return out


def kernel(x, weights, gumbel):
    nc = _get_program()
    in_maps = _shard_inputs(x, weights, gumbel)
    res = run_bass_kernel_spmd(nc, in_maps, list(range(NCORES)))
    return _unshard(res.results)


def kernel_traced(x, weights, gumbel, **trace_kwargs):
    """Like kernel() but with profiling; returns (out, BassKernelResults)."""
    nc = _get_program()
    in_maps = _shard_inputs(x, weights, gumbel)
    res = run_bass_kernel_spmd(
        nc, in_maps, list(range(NCORES)), trace=True, **trace_kwargs
    )
    return _unshard(res.results), res


# revision 15
# speedup vs baseline: 2.0918x; 2.0918x over previous
"""Trainium2 Bass kernel for the DifferentiableLayer (moe_routing) problem.

Computes out[b, o] = sum_{i,k} onehot(argmax_k(weights+gumbel))[o,i,k] * ops(x)[b,i,k]
where ops(x) = [x, sin x, cos x, tanh x, x^2, relu x] along k.

Forward value of the straight-through hard gumbel-softmax is exactly the
one-hot of argmax_k(weights + gumbel) (softmax is monotonic).

Structure: the host ships every tensor as the exact fp16 SBUF image the
kernel wants (partition-major, fully contiguous DMA), with the
contraction index i on partitions and the w/g layout [i%128, (i//128, k, o)]
so each per-chunk DMA-accumulate is a single contiguous run per
partition (cheap SWDGE descriptor generation) and every VectorE op runs
on contiguous 16-bit slabs:
  - s = w + g via SWDGE DMA accumulate, one i-chunk at a time
  - max_k via a 5-op tensor_tensor max tree over the six [128, o] slabs
  - P^T[it, k, o] = (s == m) in one broadcast compare per i-chunk
    (m broadcast over the middle k axis, innermost o stays contiguous)
  - sin/cos: xs = x/(2pi) shared prescale, then one scalar_tensor_tensor
    fold each (v = [x>=t] - xs), then ACT Sin(2pi*v + bias):
      sin(x) = Sin(2pi*([x>=0]     - x/2pi) - pi)
      cos(x) = Sin(2pi*([x>=-pi/2] - x/2pi) - 3pi/2)
    (the handful of |x| past the Sin table edge contribute O(1e-4) rel)
  - out^T[o, b] += P^T_k . ops_k^T: 96 accumulating N=512 fp16 matmuls
    at the 1 col/cycle PE streaming roofline (~216ns each warm)
fp16 for w+g keeps the argmax flip rate ~3e-4 (~3e-3 rel err measured
vs the fp32 reference; tolerance 2e-2).

A burst of N=128 scratch matmuls at t=0 warms the PE HAM clock gate
(4/8 -> 8/8) before the first real matmul issues.

Sharding: 4 batch shards x 2 out-feature shards over 8 cores.

The 64-byte engine instruction structs have a single sync-wait slot, so
cross-engine waits that would stack on one instruction are absorbed by
dependency-carrying nops, and a post-pass strips waits that are provably
dominated by an earlier wait on the same in-order queue.
"""

import numpy as np

from concourse import bass, mybir, tile
from concourse.bass import _add_dep_helper
from concourse.bass_utils import run_bass_kernel_spmd

F16 = mybir.dt.float16
F32 = mybir.dt.float32
AF = mybir.ActivationFunctionType
ALU = mybir.AluOpType

B, I, O, K = 4096, 512, 512, 6
NB, NO = 4, 2                # batch shards x out-feature shards
BL, OL = B // NB, O // NO    # 1024, 256 per core
NCORES = NB * NO

NIT = I // 128               # 4 i-chunks (contraction tiles)
NOT = OL // 128              # 2 o-tiles (psum partition tiles)
NBC = BL // 512              # 2 b-chunks (psum free tiles)
NDUMMY = 32                  # PE warm-up matmuls (N=128, ~107ns each cold)

_PI = float(np.pi)

_ENGINE_SEM = {
    "EngineType.PE": "PE",
    "EngineType.Activation": "Activation",
    "EngineType.DVE": "DVE",
}


def _strip_redundant_waits(nc: bass.Bass) -> None:
    """Drop sync waits that are dominated by an earlier wait on the same
    in-order engine queue, or (for PE/ACT/DVE) implied by the engine's own
    completion-semaphore order.  Needed because the HW instruction structs
    hold a single wait command."""
    import re

    seen = {}      # sem name -> cumulative update count
    observed = {}  # (engine, sem name) -> max wait value already waited for
    for bb in nc.main_func.blocks:
        for ins in bb.instructions:
            si = ins.sync_info
            if si is None:
                continue
            eng = str(ins.engine)
            if len(si.on_wait) >= 2:
                own = _ENGINE_SEM.get(eng)
                keep = []
                for w in si.on_wait:
                    if observed.get((eng, w.ant_name), -1) >= w.wait_value:
                        continue
                    if (
                        own is not None
                        and re.fullmatch(rf"{own}_\d+", w.ant_name)
                        and seen.get(w.ant_name, 0) >= w.wait_value
                    ):
                        continue
                    keep.append(w)
                if len(keep) != len(si.on_wait):
                    si.on_wait = keep
            for w in si.on_wait:
                key = (eng, w.ant_name)
                if observed.get(key, -1) < w.wait_value:
                    observed[key] = w.wait_value
            for u in si.on_update:
                if u.update_value is not None:
                    seen[u.ant_name] = seen.get(u.ant_name, 0) + u.update_value
    return


def _build_program() -> bass.Bass:
    nc = bass.Bass()

    # All inputs are pre-swizzled SBUF images: [128 partitions, free elems].
    xt_in = nc.declare_dram_parameter("xt", [128, NIT * BL], F16, isOutput=False)
    w_in = nc.declare_dram_parameter("w", [128, NIT * K * OL], F16, isOutput=False)
    g_in = nc.declare_dram_parameter("g", [128, NIT * K * OL], F16, isOutput=False)
    out_ext = nc.declare_dram_parameter("out", [128, NOT * BL], F16, isOutput=True)

    def dep(a, b, why):
        _add_dep_helper(a.ins, b.ins, sync=True, reason=why)

    with tile.TileContext(nc) as tc:
        with (
            tc.tile_pool(name="big", bufs=1) as big,
            tc.tile_pool(name="psum_out", bufs=1, space="PSUM") as pout,
        ):
            # ---- SBUF tiles ----
            xt_sb = big.tile([128, NIT * BL], F16)          # [p, (it, b)]
            xs_sb = big.tile([128, NIT * BL], F16)          # x/(2pi)
            s_sb = big.tile([128, NIT * K * OL], F16)       # [p, (it, k, o)] = w+g
            g_sb = big.tile([128, NIT * K * OL], F16)       # g landing buffer
            m_sb = big.tile([128, NIT * OL], F16)           # [p, (it, o)]
            pT_sb = big.tile([128, NIT * K * OL], F16)      # [p, (it, k, o)] one-hot
            tre_sb = big.tile([128, NIT * 4 * OL], F16)     # max-tree temps
            wrap_sb = big.tile([128, 2 * NIT * BL], F16)    # [p, (f, it, b)]
            ops_sb = big.tile([128, 5 * NIT * BL], F16)     # [p, (q, it, b)]
            out_sb = big.tile([128, NOT * BL], F16)         # [p, (ot, b)]
            scr_sb = big.tile([128, 128], F16)              # PE warm-up scratch
            b_sin = big.tile([128, 1], F32)                 # -pi
            b_cos = big.tile([128, 1], F32)                 # -3pi/2
            b_scl = big.tile([128, 1], F32)                 # 2pi

            xt_f = xt_sb[:]                                  # [128, 4096]
            xt_v = xt_f.rearrange("p (it b) -> p it b", it=NIT)
            xs_f = xs_sb[:]
            s_v = s_sb[:].rearrange("p (it k o) -> p it k o", k=K, it=NIT)
            s_c = s_sb[:].rearrange("p (it ko) -> p it ko", it=NIT)
            g_c = g_sb[:].rearrange("p (it ko) -> p it ko", it=NIT)
            m_v = m_sb[:].rearrange("p (it o) -> p it o", it=NIT)
            pT_v = pT_sb[:].rearrange("p (it k o) -> p it k o", k=K, it=NIT)
            tre_v = tre_sb[:].rearrange("p (it t o) -> p it t o", it=NIT, t=4)
            wrap_f = wrap_sb[:]                              # [128, 2*4096]
            ops_v = ops_sb[:].rearrange("p (q it b) -> p q it b", q=5, it=NIT)
            out_v = out_sb[:].rearrange("p (ot b) -> p ot b", ot=NOT)

            # ---- PSUM tiles ----
            po = []
            for i in range(NOT * NBC):
                po_tile = pout.tile([128, 512], F32, tag=f"po{i}")
                po.append(po_tile)
            pscr = pout.tile([128, 512], F32, tag="pscr")

            # ---- constants / warm-up ----
            scr_ms = nc.gpsimd.memset(scr_sb[:], 0.0)
            ms_sin = nc.gpsimd.memset(b_sin[:], -_PI)
            ms_cos = nc.gpsimd.memset(b_cos[:], -1.5 * _PI)
            ms_scl = nc.gpsimd.memset(b_scl[:], 2.0 * _PI)
            npe = nc.tensor.nop()
            dep(npe, scr_ms, "absorb scratch memset wait on PE")
            for d in range(NDUMMY):
                sl = (d % 4) * 128
                nc.tensor.matmul(
                    pscr[:, sl : sl + 128], scr_sb[:], scr_sb[:],
                    start=True, stop=True,
                )

            # ---- DMA loads, all on the SP HWDGE queue; g accumulated
            #      onto w via SWDGE CCE add (1 contiguous run/partition) ----
            xt_dram = xt_in[:].rearrange("p (h b) -> p h b", h=2)
            xt_hv = xt_f.rearrange("p (h b) -> p h b", h=2)
            w_dram = w_in[:].rearrange("p (it ko) -> p it ko", it=NIT)
            g_dram = g_in[:].rearrange("p (it ko) -> p it ko", it=NIT)

            wd, gd = [], []
            tail_deps = [scr_ms, ms_sin, ms_cos, ms_scl]
            # w/g interleaved on the SP HWDGE ring (FIFO completion order)
            for it in range(NIT):
                w_i = nc.sync.dma_start(out=s_c[:, it], in_=w_dram[:, it])
                g_i = nc.sync.dma_start(out=g_c[:, it], in_=g_dram[:, it])
                wd.append(w_i)
                gd.append(g_i)
                tail_deps.extend([w_i, g_i])
            # x halves on the ACT HWDGE ring (parallel to w/g)
            x_h0 = nc.scalar.dma_start(out=xt_hv[:, 0], in_=xt_dram[:, 0])
            x_h1 = nc.scalar.dma_start(out=xt_hv[:, 1], in_=xt_dram[:, 1])
            xd = [x_h0, x_h0, x_h1, x_h1]   # per-chunk alias (halves)
            tail_deps.extend([x_h0, x_h1])

            # ---- VectorE ----
            half = 2 * BL  # 2048 columns per half

            def hs(base, q, h):
                lo = q * NIT * BL + h * half
                return base[:, lo : lo + half]

            wrapS, wrapC, relu_i, sq_i, eq = {}, {}, {}, {}, {}

            def emit_wrap(h, f, thresh):
                nv = nc.vector.nop()
                dep(nv, xd[2 * h], "absorb x dma wait on DVE")
                tail_deps.append(nv)
                xsl = xt_f[:, h * half : (h + 1) * half]
                t = hs(wrap_f, f, h)
                nc.vector.tensor_scalar(
                    t, xsl, thresh, 2.0 * _PI, op0=ALU.is_ge, op1=ALU.mult
                )
                return nc.vector.tensor_sub(t, t, xsl)

            def emit_relu_sq(h):
                xsl = xt_f[:, h * half : (h + 1) * half]
                relu_i[h] = nc.vector.tensor_scalar_max(
                    hs(ops_sb[:], 4, h), xsl, 0.0
                )
                sq_i[h] = nc.vector.tensor_mul(hs(ops_sb[:], 3, h), xsl, xsl)

            def emit_mask(it):
                nv = nc.vector.nop()
                dep(nv, gd[it], "absorb g dma wait on DVE")
                tail_deps.append(nv)
                nc.vector.tensor_add(s_c[:, it], s_c[:, it], g_c[:, it])
                t = tre_v
                nc.vector.tensor_tensor(t[:, it, 0], s_v[:, it, 0], s_v[:, it, 1], op=ALU.max)
                nc.vector.tensor_tensor(t[:, it, 1], s_v[:, it, 2], s_v[:, it, 3], op=ALU.max)
                nc.vector.tensor_tensor(t[:, it, 2], s_v[:, it, 4], s_v[:, it, 5], op=ALU.max)
                nc.vector.tensor_tensor(t[:, it, 3], t[:, it, 0], t[:, it, 1], op=ALU.max)
                nc.vector.tensor_tensor(m_v[:, it], t[:, it, 2], t[:, it, 3], op=ALU.max)
                mb = m_v[:, it].unsqueeze(1).to_broadcast((128, K, OL))
                eq[it] = nc.vector.tensor_tensor(
                    pT_v[:, it], s_v[:, it], mb, op=ALU.is_equal
                )

            wrapS[0] = emit_wrap(0, 0, 0.0)
            emit_mask(0)         # needs g0
            wrapC[0] = emit_wrap(0, 1, -0.5 * _PI)
            emit_mask(1)
            wrapS[1] = emit_wrap(1, 0, 0.0)
            emit_relu_sq(0)
            emit_mask(2)
            wrapC[1] = emit_wrap(1, 1, -0.5 * _PI)
            emit_mask(3)
            emit_relu_sq(1)

            # ---- ScalarE: transcendentals per half ----
            for b in (ms_sin, ms_cos, ms_scl):
                nsc = nc.scalar.nop()
                dep(nsc, b, "absorb bias memset wait on ACT")
                tail_deps.append(nsc)
            for h in range(2):
                na = nc.scalar.nop()
                dep(na, xd[2 * h], "absorb x dma wait on ACT")
                tail_deps.append(na)
            act = {}
            act[("tanh", 0)] = nc.scalar.activation(
                hs(ops_sb[:], 2, 0), xt_f[:, 0:half], AF.Tanh
            )
            act[("sin", 0)] = nc.scalar.activation(
                hs(ops_sb[:], 0, 0), hs(wrap_f, 0, 0), AF.Sin,
                bias=b_sin[:],
            )
            act[("tanh", 1)] = nc.scalar.activation(
                hs(ops_sb[:], 2, 1), xt_f[:, half : 2 * half], AF.Tanh
            )
            act[("cos", 0)] = nc.scalar.activation(
                hs(ops_sb[:], 1, 0), hs(wrap_f, 1, 0), AF.Sin,
                bias=b_cos[:],
            )
            act[("sin", 1)] = nc.scalar.activation(
                hs(ops_sb[:], 0, 1), hs(wrap_f, 0, 1), AF.Sin,
                bias=b_sin[:],
            )
            act[("cos", 1)] = nc.scalar.activation(
                hs(ops_sb[:], 1, 1), hs(wrap_f, 1, 1), AF.Sin,
                bias=b_cos[:],
            )

            # ---- main matmuls ----
            # mask slot k (reference op order): 0=x 1=sin 2=cos 3=tanh
            # 4=sq 5=relu ; ops_v q: 0=sin 1=cos 2=tanh 3=sq 4=relu
            def rhs_src(k, it, bc):
                if k == 0:
                    return xt_v[:, it, bc * 512 : (bc + 1) * 512]
                return ops_v[:, k - 1, it, bc * 512 : (bc + 1) * 512]

            order = [
                (0, 0), (0, 3),
                (1, 0), (1, 3),
                (0, 1), (1, 1),
                (2, 0), (2, 3),
                (0, 5), (0, 4), (1, 5), (1, 4),
                (0, 2), (1, 2),
                (2, 1),
                (3, 0), (3, 3), (3, 1),
                (2, 5), (2, 4), (3, 5), (3, 4),
                (2, 2), (3, 2),
            ]
            assert len(order) == 6 * NIT
            counts = {}
            xd_absorbed = set()
            last_mm = None
            for it, k in order:
                if k == 0 and it not in xd_absorbed:
                    nx = nc.tensor.nop()
                    dep(nx, xd[it], "absorb x dma wait on PE")
                    xd_absorbed.add(it)
                for ot in range(NOT):
                    for bc in range(NBC):
                        pid = ot * NBC + bc
                        n = counts[pid] = counts.get(pid, 0) + 1
                        lhsT = pT_v[:, it, k, ot * 128 : (ot + 1) * 128]
                        last_mm = nc.tensor.matmul(
                            po[pid][:],
                            lhsT,
                            rhs_src(k, it, bc),
                            start=(n == 1),
                            stop=(n == len(order)),
                        )

            # ---- drain psums (2 on DVE, 2 on ACT — both idle by now) ----
            drains = []
            for ot in range(NOT):
                for bc in range(NBC):
                    pid = ot * NBC + bc
                    dst = out_v[:, ot, bc * 512 : (bc + 1) * 512]
                    if bc == 0:
                        d = nc.vector.tensor_copy(dst, po[pid][:])
                    else:
                        d = nc.scalar.copy(dst, po[pid][:])
                    drains.append(d)
            out_dram = out_ext[:].rearrange("p (ot b) -> p ot b", ot=NOT)
            for ot in range(NOT):
                for d in (drains[ot * NBC], drains[ot * NBC + 1]):
                    ns = nc.sync.nop()
                    dep(ns, d, "absorb drain wait before out dma")
                    tail_deps.append(ns)
                od = nc.sync.dma_start(out=out_dram[:, ot], in_=out_v[:, ot])
                tail_deps.append(od)

            # absorb outstanding completions on the SP queue so the
            # framework's tail drain ends up with only dominated waits
            tail_deps.extend(drains)
            tail_deps.append(last_mm)
            for v in (
                list(wrapS.values()) + list(wrapC.values())
                + list(relu_i.values()) + list(sq_i.values())
                + list(eq.values()) + list(act.values())
            ):
                tail_deps.append(v)
            for d in tail_deps:
                n = nc.sync.nop()
                dep(n, d, "tail wait absorb")

    _strip_redundant_waits(nc)
    return nc


_NC_CACHE = None


def _get_program():
    global _NC_CACHE
    if _NC_CACHE is None:
        _NC_CACHE = _build_program()
    return _NC_CACHE


def _shard_inputs(x, weights, gumbel):
    # x image: ximg[p, it*BL + b] = x[bs*BL + b, it*128 + p]
    xT = np.asarray(x, dtype=np.float32).T.astype(np.float16)   # [I, B]
    # w image: wimg[p, (it, k, o)] = w[o0 + o, it*128 + p, k]
    wT = np.asarray(weights, dtype=np.float32).transpose(2, 1, 0).astype(np.float16)  # [K, I, O]
    gT = np.asarray(gumbel, dtype=np.float32).transpose(2, 1, 0).astype(np.float16)

    def wimg(a, t):
        blk = a[:, :, t * OL : (t + 1) * OL]              # [K, I, OL]
        blk = blk.reshape(K, NIT, 128, OL)                # [K, it, p, o]
        return np.ascontiguousarray(
            blk.transpose(2, 1, 0, 3).reshape(128, NIT * K * OL)
        )

    def ximg(bs):
        blk = xT[:, bs * BL : (bs + 1) * BL]              # [I, BL]
        blk = blk.reshape(NIT, 128, BL)                   # [it, p, b]
        return np.ascontiguousarray(
            blk.transpose(1, 0, 2).reshape(128, NIT * BL)
        )

    wi = [wimg(wT, t) for t in range(NO)]
    gi = [wimg(gT, t) for t in range(NO)]
    xi = [ximg(bs) for bs in range(NB)]
    in_maps = []
    for c in range(NCORES):
        t, bs = divmod(c, NB)
        in_maps.append({"xt": xi[bs], "w": wi[t], "g": gi[t]})
    return in_maps


def _unshard(results):
    out = np.empty((B, O), dtype=np.float32)
    for c in range(NCORES):
        t, bs = divmod(c, NB)
        img = np.asarray(results[c]["out"])               # [128, ot*BL]
        blk = img.reshape(128, NOT, BL).transpose(1, 0, 2).reshape(OL, BL)
        out[bs * BL : (bs + 1) * BL, t * OL : (t + 1) * OL] = (
            blk.T.astype(np.float32)
        )
    return out


def kernel(x, weights, gumbel):
    nc = _get_program()
    in_maps = _shard_inputs(x, weights, gumbel)
    res = run_bass_kernel_spmd(nc, in_maps, list(range(NCORES)))
    return _unshard(res.results)


def kernel_traced(x, weights, gumbel, **trace_kwargs):
    """Like kernel() but with profiling; returns (out, BassKernelResults)."""
    nc = _get_program()
    in_maps = _shard_inputs(x, weights, gumbel)
    res = run_bass_kernel_spmd(
        nc, in_maps, list(range(NCORES)), trace=True, **trace_kwargs
    )
    return _unshard(res.results), res


# revision 16
# speedup vs baseline: 2.1647x; 1.0349x over previous
"""Trainium2 Bass kernel for the DifferentiableLayer (moe_routing) problem.

Computes out[b, o] = sum_{i,k} onehot(argmax_k(weights+gumbel))[o,i,k] * ops(x)[b,i,k]
where ops(x) = [x, sin x, cos x, tanh x, x^2, relu x] along k.

Forward value of the straight-through hard gumbel-softmax is exactly the
one-hot of argmax_k(weights + gumbel) (softmax is monotonic).

Structure: the host ships every tensor as the exact fp16 SBUF image the
kernel wants (partition-major, fully contiguous DMA), with the
contraction index i on partitions and the w/g layout [i%128, (i//128, k, o)]
so each per-chunk DMA-accumulate is a single contiguous run per
partition (cheap SWDGE descriptor generation) and every VectorE op runs
on contiguous 16-bit slabs:
  - s = w + g via SWDGE DMA accumulate, one i-chunk at a time
  - max_k via a 5-op tensor_tensor max tree over the six [128, o] slabs
  - P^T[it, k, o] = (s == m) in one broadcast compare per i-chunk
    (m broadcast over the middle k axis, innermost o stays contiguous)
  - sin/cos: xs = x/(2pi) shared prescale, then one scalar_tensor_tensor
    fold each (v = [x>=t] - xs), then ACT Sin(2pi*v + bias):
      sin(x) = Sin(2pi*([x>=0]     - x/2pi) - pi)
      cos(x) = Sin(2pi*([x>=-pi/2] - x/2pi) - 3pi/2)
    (the handful of |x| past the Sin table edge contribute O(1e-4) rel)
  - out^T[o, b] += P^T_k . ops_k^T: 96 accumulating N=512 fp16 matmuls
    at the 1 col/cycle PE streaming roofline (~216ns each warm)
fp16 for w+g keeps the argmax flip rate ~3e-4 (~3e-3 rel err measured
vs the fp32 reference; tolerance 2e-2).

A burst of N=128 scratch matmuls at t=0 warms the PE HAM clock gate
(4/8 -> 8/8) before the first real matmul issues.

Sharding: 4 batch shards x 2 out-feature shards over 8 cores.

The 64-byte engine instruction structs have a single sync-wait slot, so
cross-engine waits that would stack on one instruction are absorbed by
dependency-carrying nops, and a post-pass strips waits that are provably
dominated by an earlier wait on the same in-order queue.
"""

import numpy as np

from concourse import bass, mybir, tile
from concourse.bass import _add_dep_helper
from concourse.bass_utils import run_bass_kernel_spmd

F16 = mybir.dt.float16
F32 = mybir.dt.float32
AF = mybir.ActivationFunctionType
ALU = mybir.AluOpType

B, I, O, K = 4096, 512, 512, 6
NB, NO = 4, 2                # batch shards x out-feature shards
BL, OL = B // NB, O // NO    # 1024, 256 per core
NCORES = NB * NO

NIT = I // 128               # 4 i-chunks (contraction tiles)
NOT = OL // 128              # 2 o-tiles (psum partition tiles)
NBC = BL // 512              # 2 b-chunks (psum free tiles)
NDUMMY = 44                  # PE warm-up matmuls (N=128, ~107ns each cold)

_PI = float(np.pi)

_ENGINE_SEM = {
    "EngineType.PE": "PE",
    "EngineType.Activation": "Activation",
    "EngineType.DVE": "DVE",
}


def _strip_redundant_waits(nc: bass.Bass) -> None:
    """Drop sync waits that are dominated by an earlier wait on the same
    in-order engine queue, or (for PE/ACT/DVE) implied by the engine's own
    completion-semaphore order.  Needed because the HW instruction structs
    hold a single wait command."""
    import re

    seen = {}      # sem name -> cumulative update count
    observed = {}  # (engine, sem name) -> max wait value already waited for
    for bb in nc.main_func.blocks:
        for ins in bb.instructions:
            si = ins.sync_info
            if si is None:
                continue
            eng = str(ins.engine)
            if len(si.on_wait) >= 2:
                own = _ENGINE_SEM.get(eng)
                keep = []
                for w in si.on_wait:
                    if observed.get((eng, w.ant_name), -1) >= w.wait_value:
                        continue
                    if (
                        own is not None
                        and re.fullmatch(rf"{own}_\d+", w.ant_name)
                        and seen.get(w.ant_name, 0) >= w.wait_value
                    ):
                        continue
                    keep.append(w)
                if len(keep) != len(si.on_wait):
                    si.on_wait = keep
            for w in si.on_wait:
                key = (eng, w.ant_name)
                if observed.get(key, -1) < w.wait_value:
                    observed[key] = w.wait_value
            for u in si.on_update:
                if u.update_value is not None:
                    seen[u.ant_name] = seen.get(u.ant_name, 0) + u.update_value
    return


def _build_program() -> bass.Bass:
    nc = bass.Bass()

    # All inputs are pre-swizzled SBUF images: [128 partitions, free elems].
    xt_in = nc.declare_dram_parameter("xt", [128, NIT * BL], F16, isOutput=False)
    w_in = nc.declare_dram_parameter("w", [128, NIT * K * OL], F16, isOutput=False)
    g_in = nc.declare_dram_parameter("g", [128, NIT * K * OL], F16, isOutput=False)
    out_ext = nc.declare_dram_parameter("out", [128, NOT * BL], F16, isOutput=True)

    def dep(a, b, why):
        _add_dep_helper(a.ins, b.ins, sync=True, reason=why)

    with tile.TileContext(nc) as tc:
        with (
            tc.tile_pool(name="big", bufs=1) as big,
            tc.tile_pool(name="psum_out", bufs=1, space="PSUM") as pout,
        ):
            # ---- SBUF tiles ----
            xt_sb = big.tile([128, NIT * BL], F16)          # [p, (it, b)]
            xs_sb = big.tile([128, NIT * BL], F16)          # x/(2pi)
            s_sb = big.tile([128, NIT * K * OL], F16)       # [p, (it, k, o)] = w+g
            g_sb = big.tile([128, NIT * K * OL], F16)       # g landing buffer
            m_sb = big.tile([128, NIT * OL], F16)           # [p, (it, o)]
            pT_sb = big.tile([128, NIT * K * OL], F16)      # [p, (it, k, o)] one-hot
            tre_sb = big.tile([128, NIT * 4 * OL], F16)     # max-tree temps
            wrap_sb = big.tile([128, 2 * NIT * BL], F16)    # [p, (f, it, b)]
            ops_sb = big.tile([128, 5 * NIT * BL], F16)     # [p, (q, it, b)]
            out_sb = big.tile([128, NOT * BL], F16)         # [p, (ot, b)]
            scr_sb = big.tile([128, 128], F16)              # PE warm-up scratch
            b_sin = big.tile([128, 1], F32)                 # -pi
            b_cos = big.tile([128, 1], F32)                 # -3pi/2
            b_scl = big.tile([128, 1], F32)                 # 2pi

            xt_f = xt_sb[:]                                  # [128, 4096]
            xt_v = xt_f.rearrange("p (it b) -> p it b", it=NIT)
            xs_f = xs_sb[:]
            s_v = s_sb[:].rearrange("p (it k o) -> p it k o", k=K, it=NIT)
            s_c = s_sb[:].rearrange("p (it ko) -> p it ko", it=NIT)
            g_c = g_sb[:].rearrange("p (it ko) -> p it ko", it=NIT)
            m_v = m_sb[:].rearrange("p (it o) -> p it o", it=NIT)
            pT_v = pT_sb[:].rearrange("p (it k o) -> p it k o", k=K, it=NIT)
            tre_v = tre_sb[:].rearrange("p (it t o) -> p it t o", it=NIT, t=4)
            wrap_f = wrap_sb[:]                              # [128, 2*4096]
            ops_v = ops_sb[:].rearrange("p (q it b) -> p q it b", q=5, it=NIT)
            out_v = out_sb[:].rearrange("p (ot b) -> p ot b", ot=NOT)

            # ---- PSUM tiles ----
            po = []
            for i in range(NOT * NBC):
                po_tile = pout.tile([128, 512], F32, tag=f"po{i}")
                po.append(po_tile)
            pscr = pout.tile([128, 512], F32, tag="pscr")

            # ---- constants / warm-up ----
            scr_ms = nc.gpsimd.memset(scr_sb[:], 0.0)
            ms_sin = nc.gpsimd.memset(b_sin[:], -_PI)
            ms_cos = nc.gpsimd.memset(b_cos[:], -1.5 * _PI)
            ms_scl = nc.gpsimd.memset(b_scl[:], 2.0 * _PI)
            npe = nc.tensor.nop()
            dep(npe, scr_ms, "absorb scratch memset wait on PE")
            for d in range(NDUMMY):
                sl = (d % 4) * 128
                nc.tensor.matmul(
                    pscr[:, sl : sl + 128], scr_sb[:], scr_sb[:],
                    start=True, stop=True,
                )

            # ---- DMA loads, all on the SP HWDGE queue; g accumulated
            #      onto w via SWDGE CCE add (1 contiguous run/partition) ----
            xt_dram = xt_in[:].rearrange("p (h b) -> p h b", h=2)
            xt_hv = xt_f.rearrange("p (h b) -> p h b", h=2)
            w_dram = w_in[:].rearrange("p (it ko) -> p it ko", it=NIT)
            g_dram = g_in[:].rearrange("p (it ko) -> p it ko", it=NIT)

            wd, gd = [{}, {}, {}, {}], [{}, {}, {}, {}]
            tail_deps = [scr_ms, ms_sin, ms_cos, ms_scl]
            # SP ring (FIFO): chunk-0/1 w+g first, x halves interleaved
            wd[0] = nc.sync.dma_start(out=s_c[:, 0], in_=w_dram[:, 0])
            gd[0] = nc.sync.dma_start(out=g_c[:, 0], in_=g_dram[:, 0])
            x_h0 = nc.sync.dma_start(out=xt_hv[:, 0], in_=xt_dram[:, 0])
            wd[1] = nc.sync.dma_start(out=s_c[:, 1], in_=w_dram[:, 1])
            gd[1] = nc.sync.dma_start(out=g_c[:, 1], in_=g_dram[:, 1])
            x_h1 = nc.sync.dma_start(out=xt_hv[:, 1], in_=xt_dram[:, 1])
            # ACT ring: chunk-2/3 w+g (completes later; needed later)
            wd[2] = nc.scalar.dma_start(out=s_c[:, 2], in_=w_dram[:, 2])
            gd[2] = nc.scalar.dma_start(out=g_c[:, 2], in_=g_dram[:, 2])
            wd[3] = nc.scalar.dma_start(out=s_c[:, 3], in_=w_dram[:, 3])
            gd[3] = nc.scalar.dma_start(out=g_c[:, 3], in_=g_dram[:, 3])
            xd = [x_h0, x_h0, x_h1, x_h1]   # per-chunk alias (halves)
            tail_deps.extend([wd[0], gd[0], x_h0, wd[1], gd[1], x_h1,
                              wd[2], gd[2], wd[3], gd[3]])

            # ---- VectorE ----
            half = 2 * BL  # 2048 columns per half

            def hs(base, q, h):
                lo = q * NIT * BL + h * half
                return base[:, lo : lo + half]

            wrapS, wrapC, relu_i, sq_i, eq = {}, {}, {}, {}, {}

            def emit_wrap(h, f, thresh):
                nv = nc.vector.nop()
                dep(nv, xd[2 * h], "absorb x dma wait on DVE")
                tail_deps.append(nv)
                xsl = xt_f[:, h * half : (h + 1) * half]
                t = hs(wrap_f, f, h)
                nc.vector.tensor_scalar(
                    t, xsl, thresh, 2.0 * _PI, op0=ALU.is_ge, op1=ALU.mult
                )
                return nc.vector.tensor_sub(t, t, xsl)

            def emit_sq(h):
                xsl = xt_f[:, h * half : (h + 1) * half]
                sq_i[h] = nc.vector.tensor_mul(hs(ops_sb[:], 3, h), xsl, xsl)

            def emit_mask(it):
                nv = nc.vector.nop()
                dep(nv, gd[it], "absorb g dma wait on DVE")
                tail_deps.append(nv)
                nc.vector.tensor_add(s_c[:, it], s_c[:, it], g_c[:, it])
                t = tre_v
                nc.vector.tensor_tensor(t[:, it, 0], s_v[:, it, 0], s_v[:, it, 1], op=ALU.max)
                nc.vector.tensor_tensor(t[:, it, 1], s_v[:, it, 2], s_v[:, it, 3], op=ALU.max)
                nc.vector.tensor_tensor(t[:, it, 2], s_v[:, it, 4], s_v[:, it, 5], op=ALU.max)
                nc.vector.tensor_tensor(t[:, it, 3], t[:, it, 0], t[:, it, 1], op=ALU.max)
                nc.vector.tensor_tensor(m_v[:, it], t[:, it, 2], t[:, it, 3], op=ALU.max)
                mb = m_v[:, it].unsqueeze(1).to_broadcast((128, K, OL))
                eq[it] = nc.vector.tensor_tensor(
                    pT_v[:, it], s_v[:, it], mb, op=ALU.is_equal
                )

            emit_mask(0)         # needs g0
            wrapS[0] = emit_wrap(0, 0, 0.0)
            emit_mask(1)
            wrapC[0] = emit_wrap(0, 1, -0.5 * _PI)
            emit_mask(2)
            wrapS[1] = emit_wrap(1, 0, 0.0)
            emit_mask(3)
            wrapC[1] = emit_wrap(1, 1, -0.5 * _PI)
            emit_sq(0)
            emit_sq(1)

            # ---- ScalarE: transcendentals per half ----
            for b in (ms_sin, ms_cos, ms_scl):
                nsc = nc.scalar.nop()
                dep(nsc, b, "absorb bias memset wait on ACT")
                tail_deps.append(nsc)
            for h in range(2):
                na = nc.scalar.nop()
                dep(na, xd[2 * h], "absorb x dma wait on ACT")
                tail_deps.append(na)
            act = {}
            act[("tanh", 0)] = nc.scalar.activation(
                hs(ops_sb[:], 2, 0), xt_f[:, 0:half], AF.Tanh
            )
            act[("tanh", 1)] = nc.scalar.activation(
                hs(ops_sb[:], 2, 1), xt_f[:, half : 2 * half], AF.Tanh
            )
            act[("sin", 0)] = nc.scalar.activation(
                hs(ops_sb[:], 0, 0), hs(wrap_f, 0, 0), AF.Sin,
                bias=b_sin[:],
            )
            relu_i[0] = nc.scalar.activation(
                hs(ops_sb[:], 4, 0), xt_f[:, 0:half], AF.Relu
            )
            act[("cos", 0)] = nc.scalar.activation(
                hs(ops_sb[:], 1, 0), hs(wrap_f, 1, 0), AF.Sin,
                bias=b_cos[:],
            )
            act[("sin", 1)] = nc.scalar.activation(
                hs(ops_sb[:], 0, 1), hs(wrap_f, 0, 1), AF.Sin,
                bias=b_sin[:],
            )
            relu_i[1] = nc.scalar.activation(
                hs(ops_sb[:], 4, 1), xt_f[:, half : 2 * half], AF.Relu
            )
            act[("cos", 1)] = nc.scalar.activation(
                hs(ops_sb[:], 1, 1), hs(wrap_f, 1, 1), AF.Sin,
                bias=b_cos[:],
            )

            # ---- main matmuls ----
            # mask slot k (reference op order): 0=x 1=sin 2=cos 3=tanh
            # 4=sq 5=relu ; ops_v q: 0=sin 1=cos 2=tanh 3=sq 4=relu
            def rhs_src(k, it, bc):
                if k == 0:
                    return xt_v[:, it, bc * 512 : (bc + 1) * 512]
                return ops_v[:, k - 1, it, bc * 512 : (bc + 1) * 512]

            order = [
                (0, 0), (0, 3),
                (1, 0), (1, 3),
                (0, 1), (1, 1),
                (2, 0), (2, 3),
                (0, 2), (1, 2),
                (2, 1),
                (3, 0), (3, 3), (3, 1),
                (2, 2), (3, 2),
                (0, 5), (0, 4), (1, 5), (1, 4),
                (2, 5), (2, 4), (3, 5), (3, 4),
            ]
            assert len(order) == 6 * NIT
            counts = {}
            xd_absorbed = set()
            last_mm = None
            for it, k in order:
                if k == 0 and it not in xd_absorbed:
                    nx = nc.tensor.nop()
                    dep(nx, xd[it], "absorb x dma wait on PE")
                    xd_absorbed.add(it)
                for ot in range(NOT):
                    for bc in range(NBC):
                        pid = ot * NBC + bc
                        n = counts[pid] = counts.get(pid, 0) + 1
                        lhsT = pT_v[:, it, k, ot * 128 : (ot + 1) * 128]
                        last_mm = nc.tensor.matmul(
                            po[pid][:],
                            lhsT,
                            rhs_src(k, it, bc),
                            start=(n == 1),
                            stop=(n == len(order)),
                        )

            # ---- drain psums (2 on DVE, 2 on ACT — both idle by now) ----
            drains = []
            for ot in range(NOT):
                for bc in range(NBC):
                    pid = ot * NBC + bc
                    dst = out_v[:, ot, bc * 512 : (bc + 1) * 512]
                    if bc == 0:
                        d = nc.vector.tensor_copy(dst, po[pid][:])
                    else:
                        d = nc.scalar.copy(dst, po[pid][:])
                    drains.append(d)
            out_dram = out_ext[:].rearrange("p (ot b) -> p ot b", ot=NOT)
            for ot in range(NOT):
                for d in (drains[ot * NBC], drains[ot * NBC + 1]):
                    ns = nc.sync.nop()
                    dep(ns, d, "absorb drain wait before out dma")
                    tail_deps.append(ns)
                od = nc.sync.dma_start(out=out_dram[:, ot], in_=out_v[:, ot])
                tail_deps.append(od)

            # absorb outstanding completions on the SP queue so the
            # framework's tail drain ends up with only dominated waits
            tail_deps.extend(drains)
            tail_deps.append(last_mm)
            for v in (
                list(wrapS.values()) + list(wrapC.values())
                + list(relu_i.values()) + list(sq_i.values())
                + list(eq.values()) + list(act.values())
            ):
                tail_deps.append(v)
            for d in tail_deps:
                n = nc.sync.nop()
                dep(n, d, "tail wait absorb")

    _strip_redundant_waits(nc)
    return nc


_NC_CACHE = None


def _get_program():
    global _NC_CACHE
    if _NC_CACHE is None:
        _NC_CACHE = _build_program()
    return _NC_CACHE


def _shard_inputs(x, weights, gumbel):
    # x image: ximg[p, it*BL + b] = x[bs*BL + b, it*128 + p]
    xT = np.asarray(x, dtype=np.float32).T.astype(np.float16)   # [I, B]
    # w image: wimg[p, (it, k, o)] = w[o0 + o, it*128 + p, k]
    wT = np.asarray(weights, dtype=np.float32).transpose(2, 1, 0).astype(np.float16)  # [K, I, O]
    gT = np.asarray(gumbel, dtype=np.float32).transpose(2, 1, 0).astype(np.float16)

    def wimg(a, t):
        blk = a[:, :, t * OL : (t + 1) * OL]              # [K, I, OL]
        blk = blk.reshape(K, NIT, 128, OL)                # [K, it, p, o]
        return np.ascontiguousarray(
            blk.transpose(2, 1, 0, 3).reshape(128, NIT * K * OL)
        )

    def ximg(bs):
        blk = xT[:, bs * BL : (bs + 1) * BL]              # [I, BL]
        blk = blk.reshape(NIT, 128, BL)                   # [it, p, b]
        return np.ascontiguousarray(
            blk.transpose(1, 0, 2).reshape(128, NIT * BL)
        )

    wi = [wimg(wT, t) for t in range(NO)]
    gi = [wimg(gT, t) for t in range(NO)]
    xi = [ximg(bs) for bs in range(NB)]
    in_maps = []
    for c in range(NCORES):
        t, bs = divmod(c, NB)
        in_maps.append({"xt": xi[bs], "w": wi[t], "g": gi[t]})
    return in_maps


def _unshard(results):
    out = np.empty((B, O), dtype=np.float32)
    for c in range(NCORES):
        t, bs = divmod(c, NB)
        img = np.asarray(results[c]["out"])               # [128, ot*BL]
        blk = img.reshape(128, NOT, BL).transpose(1, 0, 2).reshape(OL, BL)
        out[bs * BL : (bs + 1) * BL, t * OL : (t + 1) * OL] = (
            blk.T.astype(np.float32)
        )
    return out


def kernel(x, weights, gumbel):
    nc = _get_program()
    in_maps = _shard_inputs(x, weights, gumbel)
    res = run_bass_kernel_spmd(nc, in_maps, list(range(NCORES)))
    return _unshard(res.results)


def kernel_traced(x, weights, gumbel, **trace_kwargs):
    """Like kernel() but with profiling; returns (out, BassKernelResults)."""
    nc = _get_program()
    in_maps = _shard_inputs(x, weights, gumbel)
    res = run_bass_kernel_spmd(
        nc, in_maps, list(range(NCORES)), trace=True, **trace_kwargs
    )
    return _unshard(res.results), res
